# revision 20
# baseline (speedup 1.0000x reference)
"""Trainium2 Bass kernel for capsule dynamic routing (nn_Capsule).

Math (per sample):
  hat[i,(n,d)] = sum_d' x[i,d'] W[d',(n,d)]        (i=1024, d'=128, n=32, d=16)
  3 routing iters: c = softmax(b, axis=n); o = squash(sum_i c[n,i] hat[i,n,:])
                   b = sum_d o[n,d] hat[i,n,d]
Never materialize hat.  W columns are permuted k' = d*32 + n so every masked
reduce is contiguous and the mask is the same [128,128] tile for every chunk.

Per group of 4 samples (stacked 4*32 = 128 partitions q=(b,n)), per iter:
  GT[d',q]   = sum_i x[i,d'] c[i,q]          (xn-chunk stationary, 32-col MMs)
  F[q,k']    = GTs^T @ wP                     (one 512-col MM)    -> s, squash
  FT_j[k',q] = wpc_j^T @ GTs                  (4 128-col MMs, const stationary)
  scale[q,1] (per-partition newton-rsqrt)  -> flip to [1,q] via identity MM
             -> scB[p,q] via K=1 ones MM   -> scMask = maskT * scB (DVE)
  tsTs       = FT * scMask                    (masked+scaled, transposed)
  HT[d',q]   = sum_j wtp_j^T @ tsTs_j         (const stationary)
  bt[i,(b,c,n)] = xT-chunk^T @ HT-cols        (i-part so softmax transposes
  e=exp(bt); z; ct=e*rz  (ct-mul on GpSimd)    nothing)
Final iter: F -> s -> o = s*scale -> out.
Sharding: data-parallel over batch, 16 samples/core x 8 cores.
"""

import os
import sys

sys.path.insert(0, "/opt/trn_rl_repo")

import numpy as np

import concourse.bass as bass
import concourse.bacc as bacc
import concourse.mybir as mybir
from concourse import tile
from concourse.bass_utils import run_bass_kernel_spmd

FP32 = mybir.dt.float32
BF16 = mybir.dt.bfloat16
I32 = mybir.dt.int32
AF = mybir.ActivationFunctionType
AX = mybir.AxisListType
AL = mybir.AluOpType

EPS = 1e-7
N_CORES = 8
B_TOTAL, IN, D = 128, 1024, 128
NCAP, DC = 32, 16
K = NCAP * DC          # 512
B_LOC = B_TOTAL // N_CORES   # 16 samples per core
GSZ = 4                # samples per group (4*32 = 128 partitions)
NG = B_LOC // GSZ      # 4 groups
NCH = IN // 128        # 8 chunks of the In dimension


def newton_scale(nc, pool, ss_in, ncols, tag):
    """scale = sqrt(v)/(0.5+v), v = ss+EPS; rsqrt = bit-trick + 2 Newton.
    ss_in: [128,ncols] fp32 AP (one col per group). Returns [128,ncols]."""
    p = pool
    sh = [128, ncols]
    ve = p.tile(sh, FP32, tag=f"{tag}ve", name=f"{tag}ve")
    nc.vector.tensor_scalar_add(ve[:], ss_in, EPS)
    ib = p.tile(sh, I32, tag=f"{tag}ib", name=f"{tag}ib")
    nc.vector.tensor_scalar(ib[:], ve[:].bitcast(I32), 1, None,
                            op0=AL.arith_shift_right)
    nc.vector.tensor_scalar(ib[:], ib[:], -1, 0x5F3759DF,
                            op0=AL.mult, op1=AL.add)
    y0 = ib[:].bitcast(FP32)
    aN = p.tile(sh, FP32, tag=f"{tag}aN", name=f"{tag}aN")
    yN = p.tile(sh, FP32, tag=f"{tag}yN", name=f"{tag}yN")
    nc.vector.tensor_mul(aN[:], y0, y0)
    nc.vector.tensor_mul(aN[:], aN[:], ve[:])
    nc.vector.tensor_scalar(aN[:], aN[:], -0.5, 1.5, op0=AL.mult, op1=AL.add)
    nc.vector.tensor_mul(yN[:], y0, aN[:])
    nc.vector.tensor_mul(aN[:], yN[:], yN[:])
    nc.vector.tensor_mul(aN[:], aN[:], ve[:])
    nc.vector.tensor_scalar(aN[:], aN[:], -0.5, 1.5, op0=AL.mult, op1=AL.add)
    nc.vector.tensor_mul(yN[:], yN[:], aN[:])   # rsqrt(ve)
    sv = p.tile(sh, FP32, tag=f"{tag}sv", name=f"{tag}sv")
    nc.vector.tensor_mul(sv[:], yN[:], ve[:])   # sqrt(ve)
    den = p.tile(sh, FP32, tag=f"{tag}dn", name=f"{tag}dn")
    nc.vector.tensor_scalar_add(den[:], ve[:], 0.5)
    rden = p.tile(sh, FP32, tag=f"{tag}rd", name=f"{tag}rd")
    nc.vector.reciprocal(rden[:], den[:])
    sc = p.tile(sh, FP32, tag=f"{tag}sc", name=f"{tag}sc")
    nc.vector.tensor_mul(sc[:], sv[:], rden[:])
    return sc


def build():
    nc = bacc.Bacc("TRN2", target_bir_lowering=False)
    xT = nc.declare_dram_parameter("xT", [B_LOC, D, IN], BF16, isOutput=False)
    xn = nc.declare_dram_parameter("xn", [B_LOC, 128, NCH, D], BF16, isOutput=False)
    wp = nc.declare_dram_parameter("wp", [D, K], BF16, isOutput=False)
    wpc = nc.declare_dram_parameter("wpc", [D, 4, 128], BF16, isOutput=False)
    wtp = nc.declare_dram_parameter("wtp", [K, D], BF16, isOutput=False)
    maskp = nc.declare_dram_parameter("maskp", [128, K], BF16, isOutput=False)
    maskt = nc.declare_dram_parameter("maskt", [128, 128], BF16, isOutput=False)
    ident = nc.declare_dram_parameter("ident", [128, 128], BF16, isOutput=False)
    out = nc.declare_dram_parameter("out", [B_LOC, NCAP, DC], FP32, isOutput=True)

    with tile.TileContext(nc) as tc:
        with (
            tc.tile_pool(name="const", bufs=1) as cpool,
            tc.tile_pool(name="xp", bufs=1) as xp,
            tc.tile_pool(name="sbp", bufs=4) as sbp,
            tc.tile_pool(name="tsp", bufs=4) as tsp,
            tc.tile_pool(name="ep", bufs=4) as ep,
            tc.tile_pool(name="ctp", bufs=8) as ctp,
            tc.tile_pool(name="small", bufs=16) as smallp,
            tc.tile_pool(name="gt", bufs=1, space="PSUM") as gtp,
            tc.tile_pool(name="fn", bufs=1, space="PSUM") as fnp,
            tc.tile_pool(name="ft", bufs=1, space="PSUM") as ftp,
            tc.tile_pool(name="sc", bufs=1, space="PSUM") as scp,
            tc.tile_pool(name="ht", bufs=1, space="PSUM") as htp,
            tc.tile_pool(name="bt", bufs=2, space="PSUM") as btp,
        ):
            wp_sb = cpool.tile([D, K], BF16, tag="wp")
            nc.sync.dma_start(wp_sb[:], wp[:])
            wpc_sb = cpool.tile([D, 4, 128], BF16, tag="wpc")
            nc.sync.dma_start(wpc_sb[:], wpc[:])
            wtp_sb = cpool.tile([128, 4, D], BF16, tag="wtp")
            nc.sync.dma_start(wtp_sb[:], wtp.rearrange("(j p) d -> p j d", p=128))
            mp_sb = cpool.tile([128, K], BF16, tag="maskp")
            nc.sync.dma_start(mp_sb[:], maskp[:])
            mt_sb = cpool.tile([128, 128], BF16, tag="maskt")
            nc.sync.dma_start(mt_sb[:], maskt[:])
            id_sb = cpool.tile([128, 128], BF16, tag="ident")
            nc.sync.dma_start(id_sb[:], ident[:])
            c0_sb = cpool.tile([128, NCAP], BF16, tag="c0")
            nc.vector.memset(c0_sb[:], 1.0 / NCAP)
            ones_row = cpool.tile([1, 128], BF16, tag="ones_row")
            nc.vector.memset(ones_row[:], 1.0)

            xT_g, xn_g = [], []
            for g in range(NG):
                t2 = xp.tile([128, GSZ, NCH, D], BF16, tag=f"xn{g}")
                t = xp.tile([128, GSZ, IN], BF16, tag=f"xT{g}")
                for b in range(GSZ):
                    bb = g * GSZ + b
                    nc.sync.dma_start(t2[:, b], xn[bb])
                    nc.sync.dma_start(t[:, b, :], xT[bb])
                xn_g.append(t2)
                xT_g.append(t)

            ct = [None] * NG

            for it in range(3):
                # ---- GT[d',q] accumulation, xn-chunk stationary ----
                GTs = []
                for g in range(NG):
                    GT4 = gtp.tile([128, 128], FP32, tag="gt4")
                    for b in range(GSZ):
                        for c in range(NCH):
                            mv = c0_sb[:] if it == 0 else ct[g][:, b, c, :]
                            nc.tensor.matmul(
                                GT4[:, 32 * b:32 * b + 32],
                                xn_g[g][:, b, c, :],
                                mv,
                                start=(c == 0),
                                stop=(c == NCH - 1),
                            )
                    Gs = sbp.tile([128, 128], BF16, tag="gts")
                    nc.scalar.copy(Gs[:], GT4[:])
                    GTs.append(Gs)

                # ---- F (n-part) -> s -> batched per-partition squash ----
                s4_l = []
                ssA = smallp.tile([128, NG], FP32, tag="ssA", name="ssA")
                for g in range(NG):
                    F4t = fnp.tile([128, 4, 128], FP32, tag="f4n", name="F4n")
                    F4 = F4t[:].rearrange("p j q -> p (j q)")
                    nc.tensor.matmul(F4, GTs[g][:], wp_sb[:],
                                     start=True, stop=True)
                    ts4 = tsp.tile([128, K], BF16, tag="ts4")
                    nc.vector.tensor_mul(ts4[:], F4, mp_sb[:])
                    s4 = smallp.tile([128, DC], FP32, tag="s4")
                    nc.vector.reduce_sum(
                        s4[:], ts4[:].rearrange("p (d n) -> p d n", d=DC),
                        axis=AX.X,
                    )
                    sq4 = smallp.tile([128, DC], FP32, tag="sq4")
                    nc.vector.tensor_mul(sq4[:], s4[:], s4[:])
                    nc.vector.reduce_sum(ssA[:, g:g + 1], sq4[:], axis=AX.X)
                    s4_l.append(s4)
                scA = newton_scale(nc, smallp, ssA[:], NG, "n")

                if it == 2:
                    for g in range(NG):
                        o4 = smallp.tile([128, DC], FP32, tag="o4")
                        nc.vector.tensor_scalar_mul(o4[:], s4_l[g][:],
                                                    scA[:, g:g + 1])
                        nc.sync.dma_start(
                            out[g * GSZ:(g + 1) * GSZ].rearrange(
                                "b n d -> (b n) d"
                            ),
                            o4[:],
                        )
                    continue

                # ---- FT chunks + scale flip/broadcast + scMask ----
                FT_l, scM_l = [], []
                for g in range(NG):
                    FT4 = ftp.tile([128, 4, 128], FP32, tag="ft4t", name="FT4")
                    for j in range(4):
                        nc.tensor.matmul(
                            FT4[:, j, :], wpc_sb[:, j, :], GTs[g][:],
                            start=True, stop=True,
                        )
                    FT_l.append(FT4)
                scAb = smallp.tile([128, NG], BF16, tag="scAb", name="scAb")
                nc.vector.tensor_scalar_mul(scAb[:], scA[:], 1.0)
                for g in range(NG):
                    scT = scp.tile([1, 128], FP32, tag="sct", name="scTps")
                    nc.tensor.matmul(scT[:], scAb[:, g:g + 1], id_sb[:],
                                     start=True, stop=True)
                    scTs = smallp.tile([1, 128], BF16, tag="scTs")
                    nc.scalar.copy(scTs[:], scT[:])
                    scB = scp.tile([128, 128], FP32, tag="scb", name="scBps")
                    nc.tensor.matmul(scB[:], ones_row[:], scTs[:],
                                     start=True, stop=True)
                    scM = sbp.tile([128, 128], BF16, tag="scm")
                    nc.vector.tensor_mul(scM[:], scB[:], mt_sb[:])
                    scM_l.append(scM)
                # ---- H^T + B + exp ----
                e4s = []
                for g in range(NG):
                    tsTs = tsp.tile([128, 4, 128], BF16, tag="tsts")
                    nc.vector.tensor_mul(
                        tsTs[:], FT_l[g][:],
                        scM_l[g][:].rearrange("p (a q) -> p a q", a=1)
                        .to_broadcast([128, 4, 128]),
                    )
                    HTu = htp.tile([128, 128], FP32, tag="htu")
                    for j in range(4):
                        nc.tensor.matmul(
                            HTu[:], wtp_sb[:, j, :], tsTs[:, j, :],
                            start=(j == 0), stop=(j == 3),
                        )
                    HTs = sbp.tile([128, 128], BF16, tag="hts")
                    nc.scalar.copy(HTs[:], HTu[:])
                    e4 = ep.tile([128, GSZ, NCH, NCAP], BF16, tag="e4")
                    for h in range(2):
                        bt2 = btp.tile([128, 2, NCH, NCAP], FP32, tag="bt2")
                        for b2 in range(2):
                            b = 2 * h + b2
                            for c in range(NCH):
                                nc.tensor.matmul(
                                    bt2[:, b2, c, :],
                                    xT_g[g][:, b, 128 * c:128 * c + 128],
                                    HTs[:, 32 * b:32 * b + 32],
                                    start=True,
                                    stop=True,
                                )
                        nc.scalar.activation(
                            e4[:, 2 * h:2 * h + 2].rearrange(
                                "p a c n -> p (a c n)"
                            ),
                            bt2[:].rearrange("p a c n -> p (a c n)"),
                            AF.Exp,
                        )
                    e4s.append(e4)
                # ---- softmax normalize (i-part; no transposes) ----
                for g in range(NG):
                    z4 = smallp.tile([128, GSZ * NCH], FP32, tag="z4")
                    nc.vector.reduce_sum(z4[:], e4s[g][:], axis=AX.X)
                    rz4 = smallp.tile([128, GSZ * NCH], BF16, tag="rz4")
                    with nc.allow_low_precision("softmax denominators O(1-30)"):
                        nc.vector.reciprocal(rz4[:], z4[:])
                    ctg = ctp.tile([128, GSZ, NCH, NCAP], BF16, tag="ct4")
                    nc.gpsimd.tensor_mul(
                        ctg[:], e4s[g][:],
                        rz4[:].rearrange("p (b c) -> p b c", b=GSZ).to_broadcast(
                            [128, GSZ, NCH, NCAP]
                        ),
                    )
                    ct[g] = ctg
    nc.compile()
    return nc


LAST_RESULT = None
_CONSTS = None


def _consts():
    global _CONSTS
    if _CONSTS is None:
        # permutation k' = d*32 + n  (k = n*16 + d)
        perm = np.empty(K, np.int64)
        for n in range(NCAP):
            for d in range(DC):
                perm[d * NCAP + n] = n * DC + d
        # maskp[p=(b,n), d*32+n'] = (n' == n)
        m32 = np.tile(np.eye(NCAP, dtype=np.float32), (1, DC)).reshape(NCAP, K)
        maskp = np.tile(m32, (GSZ, 1))
        # maskt[p, q] = (q % 32 == p % 32)
        pp, qq = np.meshgrid(np.arange(128), np.arange(128), indexing="ij")
        maskt = (pp % 32 == qq % 32).astype(np.float32)
        _CONSTS = (perm, maskp, maskt)
    return _CONSTS


def kernel(inputs, kernel):
    import ml_dtypes
    bf16 = ml_dtypes.bfloat16
    x = np.ascontiguousarray(np.asarray(inputs, dtype=np.float32))
    W = np.ascontiguousarray(np.asarray(kernel, dtype=np.float32)[0])
    xTh = np.ascontiguousarray(x.transpose(0, 2, 1).astype(bf16))
    xnL = np.ascontiguousarray(
        x.reshape(B_TOTAL, NCH, 128, D).transpose(0, 2, 1, 3).astype(bf16)
    )
    perm, maskp, maskt = _consts()
    WPf = W[:, perm]
    WP = np.ascontiguousarray(WPf.astype(bf16))
    WPC = np.ascontiguousarray(WPf.reshape(D, 4, 128).astype(bf16))
    WTP = np.ascontiguousarray(WPf.T.astype(bf16))

    nc = build()
    in_maps = [
        {
            "xT": xTh[i * B_LOC:(i + 1) * B_LOC],
            "xn": xnL[i * B_LOC:(i + 1) * B_LOC],
            "wp": WP,
            "wpc": WPC,
            "wtp": WTP,
            "maskp": maskp.astype(bf16),
            "maskt": maskt.astype(bf16),
            "ident": np.eye(128, dtype=np.float32).astype(bf16),
        }
        for i in range(N_CORES)
    ]
    res = run_bass_kernel_spmd(
        nc, in_maps, core_ids=list(range(N_CORES)),
        trace=bool(os.environ.get("KERNEL_TRACE")),
    )
    global LAST_RESULT
    LAST_RESULT = res
    return np.concatenate([res.results[i]["out"] for i in range(N_CORES)], axis=0)


if __name__ == "__main__":
    rng = np.random.default_rng(0)
    xi = rng.standard_normal((B_TOTAL, IN, D), dtype=np.float32)
    ki = (rng.standard_normal((1, D, K), dtype=np.float32) * 0.05).astype(np.float32)
    o = kernel(xi, ki)
    print(o.shape, o.dtype)


# revision 21
# speedup vs baseline: 1.0448x; 1.0448x over previous
"""Trainium2 Bass kernel for capsule dynamic routing (nn_Capsule).

Math (per sample):
  hat[i,(n,d)] = sum_d' x[i,d'] W[d',(n,d)]        (i=1024, d'=128, n=32, d=16)
  3 routing iters: c = softmax(b, axis=n); o = squash(sum_i c[n,i] hat[i,n,:])
                   b = sum_d o[n,d] hat[i,n,d]
Never materialize hat.  W columns permuted k' = d*32 + n so masked reduces are
contiguous and the mask is one [128,128] tile for every chunk.

Per group of 4 samples (stacked 4*32 = 128 partitions q=(b,n)) and iteration,
the work is split into 6 stages and EMITTED SOFTWARE-PIPELINED with a 1-stage
skew between groups, so every engine queue interleaves different stages of
different groups and nothing hard-barriers:
  S0: GT[d',q] += xn-chunk^T-stationary MMs;  GTs copy
  S1: F (512-col MM) + FT chunks;  ts4/s4/sq/ss (DVE);  tsTu = FT*maskT
  S2: newton-rsqrt scale [128,1];  flip to [1,q] (identity MM);  scB (K=1 MM)
  S3: tsTs = tsTu*scB;  HT += wtp^T MMs;  HTs copy
  S4: bt = xT-chunk MMs (i-part);  exp
  S5: z; rz; ct = e*rz (split DVE / GpSimd halves)
Final iter: S0, S1, then o = s*scale -> DMA out.
Sharding: data-parallel over batch, 16 samples/core x 8 cores.
"""

import os
import sys

sys.path.insert(0, "/opt/trn_rl_repo")

import numpy as np

import concourse.bass as bass
import concourse.bacc as bacc
import concourse.mybir as mybir
from concourse import tile
from concourse.bass_utils import run_bass_kernel_spmd

FP32 = mybir.dt.float32
BF16 = mybir.dt.bfloat16
I32 = mybir.dt.int32
AF = mybir.ActivationFunctionType
AX = mybir.AxisListType
AL = mybir.AluOpType

EPS = 1e-7
N_CORES = 8
B_TOTAL, IN, D = 128, 1024, 128
NCAP, DC = 32, 16
K = NCAP * DC
B_LOC = B_TOTAL // N_CORES
GSZ = 4
NG = B_LOC // GSZ
NCH = IN // 128


def build():
    nc = bacc.Bacc("TRN2", target_bir_lowering=False)
    xT = nc.declare_dram_parameter("xT", [B_LOC, D, IN], BF16, isOutput=False)
    xn = nc.declare_dram_parameter("xn", [B_LOC, 128, NCH, D], BF16, isOutput=False)
    wp = nc.declare_dram_parameter("wp", [D, K], BF16, isOutput=False)
    wpc = nc.declare_dram_parameter("wpc", [D, 4, 128], BF16, isOutput=False)
    wtp = nc.declare_dram_parameter("wtp", [K, D], BF16, isOutput=False)
    maskp = nc.declare_dram_parameter("maskp", [128, K], BF16, isOutput=False)
    maskt = nc.declare_dram_parameter("maskt", [128, 128], BF16, isOutput=False)
    ident = nc.declare_dram_parameter("ident", [128, 128], BF16, isOutput=False)
    out = nc.declare_dram_parameter("out", [B_LOC, NCAP, DC], FP32, isOutput=True)

    with tile.TileContext(nc) as tc:
        with (
            tc.tile_pool(name="const", bufs=1) as cpool,
            tc.tile_pool(name="xp", bufs=1) as xp,
            tc.tile_pool(name="sbp", bufs=4) as sbp,
            tc.tile_pool(name="tsp", bufs=4) as tsp,
            tc.tile_pool(name="ep", bufs=4) as ep,
            tc.tile_pool(name="ctp", bufs=8) as ctp,
            tc.tile_pool(name="small", bufs=16) as smallp,
            tc.tile_pool(name="gt", bufs=1, space="PSUM") as gtp,
            tc.tile_pool(name="fn", bufs=1, space="PSUM") as fnp,
            tc.tile_pool(name="ft", bufs=1, space="PSUM") as ftp,
            tc.tile_pool(name="sc", bufs=1, space="PSUM") as scp,
            tc.tile_pool(name="ht", bufs=1, space="PSUM") as htp,
            tc.tile_pool(name="bt", bufs=2, space="PSUM") as btp,
        ):
            wp_sb = cpool.tile([D, K], BF16, tag="wp")
            nc.sync.dma_start(wp_sb[:], wp[:])
            wpc_sb = cpool.tile([D, 4, 128], BF16, tag="wpc")
            nc.sync.dma_start(wpc_sb[:], wpc[:])
            wtp_sb = cpool.tile([128, 4, D], BF16, tag="wtp")
            nc.sync.dma_start(wtp_sb[:], wtp.rearrange("(j p) d -> p j d", p=128))
            mp_sb = cpool.tile([128, K], BF16, tag="maskp")
            nc.sync.dma_start(mp_sb[:], maskp[:])
            mt_sb = cpool.tile([128, 128], BF16, tag="maskt")
            nc.sync.dma_start(mt_sb[:], maskt[:])
            id_sb = cpool.tile([128, 128], BF16, tag="ident")
            nc.sync.dma_start(id_sb[:], ident[:])
            c0_sb = cpool.tile([128, NCAP], BF16, tag="c0")
            nc.vector.memset(c0_sb[:], 1.0 / NCAP)
            ones_row = cpool.tile([1, 128], BF16, tag="ones_row")
            nc.vector.memset(ones_row[:], 1.0)

            xT_g, xn_g = [], []
            for g in range(NG):
                t2 = xp.tile([128, GSZ, NCH, D], BF16, tag=f"xn{g}")
                t = xp.tile([128, GSZ, IN], BF16, tag=f"xT{g}")
                for b in range(GSZ):
                    bb = g * GSZ + b
                    nc.sync.dma_start(t2[:, b], xn[bb])
                    nc.sync.dma_start(t[:, b, :], xT[bb])
                xn_g.append(t2)
                xT_g.append(t)

            # persistent cross-stage state, per group
            st = [dict() for _ in range(NG)]
            ct = [None] * NG

            def s0(g, it):
                GT4 = gtp.tile([128, 128], FP32, tag="gt4")
                for b in range(GSZ):
                    for c in range(NCH):
                        mv = c0_sb[:] if it == 0 else ct[g][:, b, c, :]
                        nc.tensor.matmul(
                            GT4[:, 32 * b:32 * b + 32],
                            xn_g[g][:, b, c, :],
                            mv,
                            start=(c == 0),
                            stop=(c == NCH - 1),
                        )
                Gs = sbp.tile([128, 128], BF16, tag="gts")
                nc.scalar.copy(Gs[:], GT4[:])
                st[g]["GTs"] = Gs

            def s1(g, it):
                Gs = st[g].pop("GTs")
                F4t = fnp.tile([128, 4, 128], FP32, tag="f4n", name="F4n")
                F4 = F4t[:].rearrange("p j q -> p (j q)")
                nc.tensor.matmul(F4, Gs[:], wp_sb[:], start=True, stop=True)
                if it < 2:
                    FT4 = ftp.tile([128, 4, 128], FP32, tag="ft4t", name="FT4")
                    for j in range(4):
                        nc.tensor.matmul(
                            FT4[:, j, :], wpc_sb[:, j, :], Gs[:],
                            start=True, stop=True,
                        )
                    tsTu = tsp.tile([128, 4, 128], BF16, tag="tstu")
                    nc.vector.tensor_mul(
                        tsTu[:], FT4[:],
                        mt_sb[:].rearrange("p (a q) -> p a q", a=1)
                        .to_broadcast([128, 4, 128]),
                    )
                    st[g]["tsTu"] = tsTu
                ts4 = tsp.tile([128, K], BF16, tag="ts4")
                nc.vector.tensor_mul(ts4[:], F4, mp_sb[:])
                s4 = smallp.tile([128, DC], FP32, tag="s4")
                nc.vector.reduce_sum(
                    s4[:], ts4[:].rearrange("p (d n) -> p d n", d=DC), axis=AX.X
                )
                sq4 = smallp.tile([128, DC], FP32, tag="sq4")
                nc.vector.tensor_mul(sq4[:], s4[:], s4[:])
                ss4 = smallp.tile([128, 1], FP32, tag="ss4")
                nc.vector.reduce_sum(ss4[:], sq4[:], axis=AX.X)
                st[g]["s4"] = s4
                st[g]["ss4"] = ss4

            def s2(g, it):
                # newton-rsqrt scale, per-partition [128,1]
                ss4 = st[g].pop("ss4")
                p = smallp
                ve = p.tile([128, 1], FP32, tag="ve")
                nc.vector.tensor_scalar_add(ve[:], ss4[:], EPS)
                ib = p.tile([128, 1], I32, tag="ib")
                nc.vector.tensor_scalar(ib[:], ve[:].bitcast(I32), 1, None,
                                        op0=AL.arith_shift_right)
                nc.vector.tensor_scalar(ib[:], ib[:], -1, 0x5F3759DF,
                                        op0=AL.mult, op1=AL.add)
                y0 = ib[:].bitcast(FP32)
                aN = p.tile([128, 1], FP32, tag="aN")
                yN = p.tile([128, 1], FP32, tag="yN")
                nc.vector.tensor_mul(aN[:], y0, y0)
                nc.vector.tensor_mul(aN[:], aN[:], ve[:])
                nc.vector.tensor_scalar(aN[:], aN[:], -0.5, 1.5,
                                        op0=AL.mult, op1=AL.add)
                nc.vector.tensor_mul(yN[:], y0, aN[:])
                nc.vector.tensor_mul(aN[:], yN[:], yN[:])
                nc.vector.tensor_mul(aN[:], aN[:], ve[:])
                nc.vector.tensor_scalar(aN[:], aN[:], -0.5, 1.5,
                                        op0=AL.mult, op1=AL.add)
                nc.vector.tensor_mul(yN[:], yN[:], aN[:])
                sv = p.tile([128, 1], FP32, tag="sv")
                nc.vector.tensor_mul(sv[:], yN[:], ve[:])
                den = p.tile([128, 1], FP32, tag="den")
                nc.vector.tensor_scalar_add(den[:], ve[:], 0.5)
                rden = p.tile([128, 1], FP32, tag="rden")
                nc.vector.reciprocal(rden[:], den[:])
                sc4 = p.tile([128, 1], FP32, tag="sc4")
                nc.vector.tensor_mul(sc4[:], sv[:], rden[:])
                if it == 2:
                    o4 = p.tile([128, DC], FP32, tag="o4")
                    nc.vector.tensor_scalar_mul(o4[:], st[g].pop("s4")[:],
                                                sc4[:])
                    nc.sync.dma_start(
                        out[g * GSZ:(g + 1) * GSZ].rearrange("b n d -> (b n) d"),
                        o4[:],
                    )
                    return
                st[g].pop("s4")
                sc4b = p.tile([128, 1], BF16, tag="sc4b")
                nc.vector.tensor_scalar_mul(sc4b[:], sc4[:], 1.0)
                scT = scp.tile([1, 128], FP32, tag="sct", name="scTps")
                nc.tensor.matmul(scT[:], sc4b[:], id_sb[:], start=True,
                                 stop=True)
                scTs = p.tile([1, 128], BF16, tag="scTs")
                nc.scalar.copy(scTs[:], scT[:])
                scB = scp.tile([128, 128], FP32, tag="scb", name="scBps")
                nc.tensor.matmul(scB[:], ones_row[:], scTs[:], start=True,
                                 stop=True)
                scBs = sbp.tile([128, 128], BF16, tag="scbs")
                nc.scalar.copy(scBs[:], scB[:])
                st[g]["scBs"] = scBs

            def s3(g, it):
                tsTu = st[g].pop("tsTu")
                scBs = st[g].pop("scBs")
                tsTs = tsp.tile([128, 4, 128], BF16, tag="tsts")
                nc.vector.tensor_mul(
                    tsTs[:], tsTu[:],
                    scBs[:].rearrange("p (a q) -> p a q", a=1)
                    .to_broadcast([128, 4, 128]),
                )
                HTu = htp.tile([128, 128], FP32, tag="htu")
                for j in range(4):
                    nc.tensor.matmul(
                        HTu[:], wtp_sb[:, j, :], tsTs[:, j, :],
                        start=(j == 0), stop=(j == 3),
                    )
                HTs = sbp.tile([128, 128], BF16, tag="hts")
                nc.scalar.copy(HTs[:], HTu[:])
                st[g]["HTs"] = HTs

            def s4stage(g, it):
                HTs = st[g].pop("HTs")
                e4 = ep.tile([128, GSZ, NCH, NCAP], BF16, tag="e4")
                for h in range(2):
                    bt2 = btp.tile([128, 2, NCH, NCAP], FP32, tag="bt2")
                    for b2 in range(2):
                        b = 2 * h + b2
                        for c in range(NCH):
                            nc.tensor.matmul(
                                bt2[:, b2, c, :],
                                xT_g[g][:, b, 128 * c:128 * c + 128],
                                HTs[:, 32 * b:32 * b + 32],
                                start=True,
                                stop=True,
                            )
                    nc.scalar.activation(
                        e4[:, 2 * h:2 * h + 2].rearrange("p a c n -> p (a c n)"),
                        bt2[:].rearrange("p a c n -> p (a c n)"),
                        AF.Exp,
                    )
                st[g]["e4"] = e4

            def s5(g, it):
                e4 = st[g].pop("e4")
                z4 = smallp.tile([128, GSZ * NCH], FP32, tag="z4")
                nc.vector.reduce_sum(z4[:], e4[:], axis=AX.X)
                rz4 = smallp.tile([128, GSZ * NCH], BF16, tag="rz4")
                with nc.allow_low_precision("softmax denominators O(1-30)"):
                    nc.vector.reciprocal(rz4[:], z4[:])
                ctg = ctp.tile([128, GSZ, NCH, NCAP], BF16, tag="ct4")
                rzv = rz4[:].rearrange("p (b c) -> p b c", b=GSZ)
                nc.vector.tensor_mul(
                    ctg[:, 0:2], e4[:, 0:2],
                    rzv[:, 0:2].to_broadcast([128, 2, NCH, NCAP]),
                )
                nc.gpsimd.tensor_mul(
                    ctg[:, 2:4], e4[:, 2:4],
                    rzv[:, 2:4].to_broadcast([128, 2, NCH, NCAP]),
                )
                ct[g] = ctg

            # stage list per group: 3 iterations, last one truncated
            STAGES = []
            for it in range(2):
                STAGES += [(s0, it), (s1, it), (s2, it), (s3, it),
                           (s4stage, it), (s5, it)]
            STAGES += [(s0, 2), (s1, 2), (s2, 2)]

            NS = len(STAGES)
            for r in range(NS + NG - 1):
                for g in range(NG):
                    s = r - g
                    if 0 <= s < NS:
                        fn, it = STAGES[s]
                        fn(g, it)
    nc.compile()
    return nc


LAST_RESULT = None
_CONSTS = None


def _consts():
    global _CONSTS
    if _CONSTS is None:
        perm = np.empty(K, np.int64)
        for n in range(NCAP):
            for d in range(DC):
                perm[d * NCAP + n] = n * DC + d
        m32 = np.tile(np.eye(NCAP, dtype=np.float32), (1, DC)).reshape(NCAP, K)
        maskp = np.tile(m32, (GSZ, 1))
        pp, qq = np.meshgrid(np.arange(128), np.arange(128), indexing="ij")
        maskt = (pp % 32 == qq % 32).astype(np.float32)
        _CONSTS = (perm, maskp, maskt)
    return _CONSTS


def kernel(inputs, kernel):
    import ml_dtypes
    bf16 = ml_dtypes.bfloat16
    x = np.ascontiguousarray(np.asarray(inputs, dtype=np.float32))
    W = np.ascontiguousarray(np.asarray(kernel, dtype=np.float32)[0])
    xTh = np.ascontiguousarray(x.transpose(0, 2, 1).astype(bf16))
    xnL = np.ascontiguousarray(
        x.reshape(B_TOTAL, NCH, 128, D).transpose(0, 2, 1, 3).astype(bf16)
    )
    perm, maskp, maskt = _consts()
    WPf = W[:, perm]
    WP = np.ascontiguousarray(WPf.astype(bf16))
    WPC = np.ascontiguousarray(WPf.reshape(D, 4, 128).astype(bf16))
    WTP = np.ascontiguousarray(WPf.T.astype(bf16))

    nc = build()
    in_maps = [
        {
            "xT": xTh[i * B_LOC:(i + 1) * B_LOC],
            "xn": xnL[i * B_LOC:(i + 1) * B_LOC],
            "wp": WP,
            "wpc": WPC,
            "wtp": WTP,
            "maskp": maskp.astype(bf16),
            "maskt": maskt.astype(bf16),
            "ident": np.eye(128, dtype=np.float32).astype(bf16),
        }
        for i in range(N_CORES)
    ]
    res = run_bass_kernel_spmd(
        nc, in_maps, core_ids=list(range(N_CORES)),
        trace=bool(os.environ.get("KERNEL_TRACE")),
    )
    global LAST_RESULT
    LAST_RESULT = res
    return np.concatenate([res.results[i]["out"] for i in range(N_CORES)], axis=0)


if __name__ == "__main__":
    rng = np.random.default_rng(0)
    xi = rng.standard_normal((B_TOTAL, IN, D), dtype=np.float32)
    ki = (rng.standard_normal((1, D, K), dtype=np.float32) * 0.05).astype(np.float32)
    o = kernel(xi, ki)
    print(o.shape, o.dtype)


# revision 29
# speedup vs baseline: 1.0538x; 1.0086x over previous
"""Trainium2 Bass kernel for capsule dynamic routing (nn_Capsule).

Math (per sample):
  hat[i,(n,d)] = sum_d' x[i,d'] W[d',(n,d)]        (i=1024, d'=128, n=32, d=16)
  3 routing iters: c = softmax(b, axis=n); o = squash(sum_i c[n,i] hat[i,n,:])
                   b = sum_d o[n,d] hat[i,n,d]
Never materialize hat.  W columns permuted k' = d*32 + n so masked reduces are
contiguous and the mask is one [128,128] tile for every chunk.

Per group of 4 samples (stacked 4*32 = 128 partitions q=(b,n)) and iteration,
the work is split into 6 stages and EMITTED SOFTWARE-PIPELINED with a 1-stage
skew between groups, so every engine queue interleaves different stages of
different groups and nothing hard-barriers:
  S0: GT[d',q] += xn-chunk^T-stationary MMs;  GTs copy
  S1: F (512-col MM) + FT chunks;  ts4/s4/sq/ss (DVE);  tsTu = FT*maskT
  S2: newton-rsqrt scale [128,1];  flip to [1,q] (identity MM);  scB (K=1 MM)
  S3: tsTs = tsTu*scB;  HT += wtp^T MMs;  HTs copy
  S4: bt = xT-chunk MMs (i-part);  exp
  S5: z; rz; ct = e*rz (split DVE / GpSimd halves)
Final iter: S0, S1, then o = s*scale -> DMA out.
Sharding: data-parallel over batch, 16 samples/core x 8 cores.
"""

import os
import sys

sys.path.insert(0, "/opt/trn_rl_repo")

import numpy as np

import concourse.bass as bass
import concourse.bacc as bacc
import concourse.mybir as mybir
from concourse import tile
from concourse.bass_utils import run_bass_kernel_spmd

FP32 = mybir.dt.float32
BF16 = mybir.dt.bfloat16
I32 = mybir.dt.int32
AF = mybir.ActivationFunctionType
AX = mybir.AxisListType
AL = mybir.AluOpType

EPS = 1e-7
N_CORES = 8
B_TOTAL, IN, D = 128, 1024, 128
NCAP, DC = 32, 16
K = NCAP * DC
B_LOC = B_TOTAL // N_CORES
GSZ = 4
NG = B_LOC // GSZ
NCH = IN // 128


def build():
    nc = bacc.Bacc("TRN2", target_bir_lowering=False)
    xT = nc.declare_dram_parameter("xT", [B_LOC, D, IN], BF16, isOutput=False)
    xn = nc.declare_dram_parameter("xn", [B_LOC, 128, NCH, D], BF16, isOutput=False)
    wp = nc.declare_dram_parameter("wp", [D, K], BF16, isOutput=False)
    wpc = nc.declare_dram_parameter("wpc", [D, 4, 128], BF16, isOutput=False)
    wtp = nc.declare_dram_parameter("wtp", [K, D], BF16, isOutput=False)
    maskp = nc.declare_dram_parameter("maskp", [128, K], BF16, isOutput=False)
    maskt = nc.declare_dram_parameter("maskt", [128, 128], BF16, isOutput=False)
    ident = nc.declare_dram_parameter("ident", [128, 128], BF16, isOutput=False)
    out = nc.declare_dram_parameter("out", [B_LOC, NCAP, DC], FP32, isOutput=True)

    with tile.TileContext(nc) as tc:
        with (
            tc.tile_pool(name="const", bufs=1) as cpool,
            tc.tile_pool(name="xp", bufs=1) as xp,
            tc.tile_pool(name="sbp", bufs=4) as sbp,
            tc.tile_pool(name="tsp", bufs=4) as tsp,
            tc.tile_pool(name="ep", bufs=4) as ep,
            tc.tile_pool(name="ctp", bufs=8) as ctp,
            tc.tile_pool(name="small", bufs=16) as smallp,
            tc.tile_pool(name="gt", bufs=1, space="PSUM") as gtp,
            tc.tile_pool(name="fn", bufs=1, space="PSUM") as fnp,
            tc.tile_pool(name="ft", bufs=1, space="PSUM") as ftp,
            tc.tile_pool(name="sc", bufs=1, space="PSUM") as scp,
            tc.tile_pool(name="ht", bufs=1, space="PSUM") as htp,
            tc.tile_pool(name="bt", bufs=2, space="PSUM") as btp,
        ):
            wp_sb = cpool.tile([D, K], BF16, tag="wp")
            nc.sync.dma_start(wp_sb[:], wp[:])
            wpc_sb = cpool.tile([D, 4, 128], BF16, tag="wpc")
            nc.sync.dma_start(wpc_sb[:], wpc[:])
            wtp_sb = cpool.tile([128, 4, D], BF16, tag="wtp")
            nc.sync.dma_start(wtp_sb[:], wtp.rearrange("(j p) d -> p j d", p=128))
            mp_sb = cpool.tile([128, K], BF16, tag="maskp")
            nc.sync.dma_start(mp_sb[:], maskp[:])
            mt_sb = cpool.tile([128, 128], BF16, tag="maskt")
            nc.sync.dma_start(mt_sb[:], maskt[:])
            id_sb = cpool.tile([128, 128], BF16, tag="ident")
            nc.sync.dma_start(id_sb[:], ident[:])
            c0_sb = cpool.tile([128, NCAP], BF16, tag="c0")
            nc.vector.memset(c0_sb[:], 1.0 / NCAP)
            ones_row = cpool.tile([1, 128], BF16, tag="ones_row")
            nc.vector.memset(ones_row[:], 1.0)

            xT_g, xn_g = [], []
            for g in range(NG):
                t2 = xp.tile([128, GSZ, NCH, D], BF16, tag=f"xn{g}")
                t = xp.tile([128, GSZ, IN], BF16, tag=f"xT{g}")
                for b in range(GSZ):
                    bb = g * GSZ + b
                    nc.sync.dma_start(t2[:, b], xn[bb])
                    nc.sync.dma_start(t[:, b, :], xT[bb])
                xn_g.append(t2)
                xT_g.append(t)

            # persistent cross-stage state, per group
            st = [dict() for _ in range(NG)]
            ct = [None] * NG

            def s0(g, it):
                GT4 = gtp.tile([128, 128], FP32, tag="gt4")
                for b in range(GSZ):
                    for c in range(NCH):
                        mv = c0_sb[:] if it == 0 else ct[g][:, b, c, :]
                        nc.tensor.matmul(
                            GT4[:, 32 * b:32 * b + 32],
                            xn_g[g][:, b, c, :],
                            mv,
                            start=(c == 0),
                            stop=(c == NCH - 1),
                        )
                Gs = sbp.tile([128, 128], BF16, tag="gts")
                nc.scalar.copy(Gs[:], GT4[:])
                st[g]["GTs"] = Gs

            def s1(g, it):
                Gs = st[g].pop("GTs")
                F4t = fnp.tile([128, 4, 128], FP32, tag="f4n", name="F4n")
                F4 = F4t[:].rearrange("p j q -> p (j q)")
                nc.tensor.matmul(F4, Gs[:], wp_sb[:], start=True, stop=True)
                if it < 2:
                    FT4 = ftp.tile([128, 4, 128], FP32, tag="ft4t", name="FT4")
                    for j in range(4):
                        nc.tensor.matmul(
                            FT4[:, j, :], wpc_sb[:, j, :], Gs[:],
                            start=True, stop=True,
                        )
                    tsTu = tsp.tile([128, 4, 128], BF16, tag="tstu")
                    nc.vector.tensor_mul(
                        tsTu[:], FT4[:],
                        mt_sb[:].rearrange("p (a q) -> p a q", a=1)
                        .to_broadcast([128, 4, 128]),
                    )
                    st[g]["tsTu"] = tsTu
                ts4 = tsp.tile([128, K], BF16, tag="ts4")
                nc.vector.tensor_mul(ts4[:], F4, mp_sb[:])
                s4 = smallp.tile([128, DC], FP32, tag="s4")
                nc.vector.reduce_sum(
                    s4[:], ts4[:].rearrange("p (d n) -> p d n", d=DC), axis=AX.X
                )
                sq4 = smallp.tile([128, DC], FP32, tag="sq4")
                nc.vector.tensor_mul(sq4[:], s4[:], s4[:])
                ss4 = smallp.tile([128, 1], FP32, tag="ss4")
                nc.vector.reduce_sum(ss4[:], sq4[:], axis=AX.X)
                st[g]["s4"] = s4
                st[g]["ss4"] = ss4

            def s2(g, it):
                # newton-rsqrt scale, per-partition [128,1]
                ss4 = st[g].pop("ss4")
                p = smallp
                ve = p.tile([128, 1], FP32, tag="ve")
                nc.vector.tensor_scalar_add(ve[:], ss4[:], EPS)
                ib = p.tile([128, 1], I32, tag="ib")
                nc.vector.tensor_scalar(ib[:], ve[:].bitcast(I32), 1, None,
                                        op0=AL.arith_shift_right)
                nc.vector.tensor_scalar(ib[:], ib[:], -1, 0x5F3759DF,
                                        op0=AL.mult, op1=AL.add)
                y0 = ib[:].bitcast(FP32)
                aN = p.tile([128, 1], FP32, tag="aN")
                yN = p.tile([128, 1], FP32, tag="yN")
                nc.vector.tensor_mul(aN[:], y0, y0)
                nc.vector.tensor_mul(aN[:], aN[:], ve[:])
                nc.vector.tensor_scalar(aN[:], aN[:], -0.5, 1.5,
                                        op0=AL.mult, op1=AL.add)
                nc.vector.tensor_mul(yN[:], y0, aN[:])
                sv = p.tile([128, 1], FP32, tag="sv")
                nc.vector.tensor_mul(sv[:], yN[:], ve[:])
                den = p.tile([128, 1], FP32, tag="den")
                nc.vector.tensor_scalar_add(den[:], ve[:], 0.5)
                rden = p.tile([128, 1], FP32, tag="rden")
                nc.vector.reciprocal(rden[:], den[:])
                if it == 2:
                    sc4 = p.tile([128, 1], FP32, tag="sc4")
                    nc.vector.tensor_mul(sc4[:], sv[:], rden[:])
                    o4 = p.tile([128, DC], FP32, tag="o4")
                    nc.vector.tensor_scalar_mul(o4[:], st[g].pop("s4")[:],
                                                sc4[:])
                    nc.sync.dma_start(
                        out[g * GSZ:(g + 1) * GSZ].rearrange("b n d -> (b n) d"),
                        o4[:],
                    )
                    return
                st[g].pop("s4")
                sc4b = p.tile([128, 1], BF16, tag="sc4b")
                nc.vector.tensor_scalar_mul(sc4b[:], sv[:], rden[:])
                scT = scp.tile([1, 128], FP32, tag="sct", name="scTps")
                nc.tensor.matmul(scT[:], sc4b[:], id_sb[:], start=True,
                                 stop=True)
                scTs = p.tile([1, 128], BF16, tag="scTs")
                nc.scalar.copy(scTs[:], scT[:])
                scB = scp.tile([128, 128], FP32, tag="scb", name="scBps")
                nc.tensor.matmul(scB[:], ones_row[:], scTs[:], start=True,
                                 stop=True)
                scBs = sbp.tile([128, 128], BF16, tag="scbs")
                nc.scalar.copy(scBs[:], scB[:])
                st[g]["scBs"] = scBs

            def s3(g, it):
                tsTu = st[g].pop("tsTu")
                scBs = st[g].pop("scBs")
                tsTs = tsp.tile([128, 4, 128], BF16, tag="tsts")
                nc.vector.tensor_mul(
                    tsTs[:], tsTu[:],
                    scBs[:].rearrange("p (a q) -> p a q", a=1)
                    .to_broadcast([128, 4, 128]),
                )
                HTu = htp.tile([128, 128], FP32, tag="htu")
                for j in range(4):
                    nc.tensor.matmul(
                        HTu[:], wtp_sb[:, j, :], tsTs[:, j, :],
                        start=(j == 0), stop=(j == 3),
                    )
                HTs = sbp.tile([128, 128], BF16, tag="hts")
                nc.scalar.copy(HTs[:], HTu[:])
                st[g]["HTs"] = HTs

            def s4stage(g, it):
                HTs = st[g].pop("HTs")
                e4 = ep.tile([128, GSZ, NCH, NCAP], BF16, tag="e4")
                for h in range(2):
                    bt2 = btp.tile([128, 2, NCH, NCAP], FP32, tag="bt2")
                    for b2 in range(2):
                        b = 2 * h + b2
                        for c in range(NCH):
                            nc.tensor.matmul(
                                bt2[:, b2, c, :],
                                xT_g[g][:, b, 128 * c:128 * c + 128],
                                HTs[:, 32 * b:32 * b + 32],
                                start=True,
                                stop=True,
                            )
                    nc.scalar.activation(
                        e4[:, 2 * h:2 * h + 2].rearrange("p a c n -> p (a c n)"),
                        bt2[:].rearrange("p a c n -> p (a c n)"),
                        AF.Exp,
                    )
                st[g]["e4"] = e4

            def s5(g, it):
                e4 = st[g].pop("e4")
                z4 = smallp.tile([128, GSZ * NCH], FP32, tag="z4")
                nc.vector.reduce_sum(z4[:], e4[:], axis=AX.X)
                rz4 = smallp.tile([128, GSZ * NCH], BF16, tag="rz4")
                with nc.allow_low_precision("softmax denominators O(1-30)"):
                    nc.vector.reciprocal(rz4[:], z4[:])
                ctg = ctp.tile([128, GSZ, NCH, NCAP], BF16, tag="ct4")
                rzv = rz4[:].rearrange("p (b c) -> p b c", b=GSZ)
                nc.gpsimd.tensor_mul(
                    ctg[:], e4[:],
                    rzv[:].to_broadcast([128, GSZ, NCH, NCAP]),
                )
                ct[g] = ctg

            # stage list per group: 3 iterations, last one truncated
            STAGES = []
            for it in range(2):
                STAGES += [(s0, it), (s1, it), (s2, it), (s3, it),
                           (s4stage, it), (s5, it)]
            STAGES += [(s0, 2), (s1, 2), (s2, 2)]

            NS = len(STAGES)
            for r in range(NS + NG - 1):
                for g in range(NG):
                    s = r - g
                    if 0 <= s < NS:
                        fn, it = STAGES[s]
                        fn(g, it)
    nc.compile()
    return nc


LAST_RESULT = None
_CONSTS = None


def _consts():
    global _CONSTS
    if _CONSTS is None:
        perm = np.empty(K, np.int64)
        for n in range(NCAP):
            for d in range(DC):
                perm[d * NCAP + n] = n * DC + d
        m32 = np.tile(np.eye(NCAP, dtype=np.float32), (1, DC)).reshape(NCAP, K)
        maskp = np.tile(m32, (GSZ, 1))
        pp, qq = np.meshgrid(np.arange(128), np.arange(128), indexing="ij")
        maskt = (pp % 32 == qq % 32).astype(np.float32)
        _CONSTS = (perm, maskp, maskt)
    return _CONSTS


def kernel(inputs, kernel):
    import ml_dtypes
    bf16 = ml_dtypes.bfloat16
    x = np.ascontiguousarray(np.asarray(inputs, dtype=np.float32))
    W = np.ascontiguousarray(np.asarray(kernel, dtype=np.float32)[0])
    xTh = np.ascontiguousarray(x.transpose(0, 2, 1).astype(bf16))
    xnL = np.ascontiguousarray(
        x.reshape(B_TOTAL, NCH, 128, D).transpose(0, 2, 1, 3).astype(bf16)
    )
    perm, maskp, maskt = _consts()
    WPf = W[:, perm]
    WP = np.ascontiguousarray(WPf.astype(bf16))
    WPC = np.ascontiguousarray(WPf.reshape(D, 4, 128).astype(bf16))
    WTP = np.ascontiguousarray(WPf.T.astype(bf16))

    nc = build()
    in_maps = [
        {
            "xT": xTh[i * B_LOC:(i + 1) * B_LOC],
            "xn": xnL[i * B_LOC:(i + 1) * B_LOC],
            "wp": WP,
            "wpc": WPC,
            "wtp": WTP,
            "maskp": maskp.astype(bf16),
            "maskt": maskt.astype(bf16),
            "ident": np.eye(128, dtype=np.float32).astype(bf16),
        }
        for i in range(N_CORES)
    ]
    res = run_bass_kernel_spmd(
        nc, in_maps, core_ids=list(range(N_CORES)),
        trace=bool(os.environ.get("KERNEL_TRACE")),
    )
    global LAST_RESULT
    LAST_RESULT = res
    return np.concatenate([res.results[i]["out"] for i in range(N_CORES)], axis=0)


if __name__ == "__main__":
    rng = np.random.default_rng(0)
    xi = rng.standard_normal((B_TOTAL, IN, D), dtype=np.float32)
    ki = (rng.standard_normal((1, D, K), dtype=np.float32) * 0.05).astype(np.float32)
    o = kernel(xi, ki)
    print(o.shape, o.dtype)


# revision 31
# speedup vs baseline: 1.2247x; 1.1622x over previous
"""Trainium2 Bass kernel for capsule dynamic routing (nn_Capsule).

Math (per sample):
  hat[i,(n,d)] = sum_d' x[i,d'] W[d',(n,d)]        (i=1024, d'=128, n=32, d=16)
  3 routing iters: c = softmax(b, axis=n); o = squash(sum_i c[n,i] hat[i,n,:])
                   b = sum_d o[n,d] hat[i,n,d]
Never materialize hat.  W columns permuted k' = d*32 + n so masked reduces are
contiguous and the mask is one [128,128] tile for every chunk.

Per group of 4 samples (stacked 4*32 = 128 partitions q=(b,n)) and iteration,
the work is split into 6 stages and EMITTED SOFTWARE-PIPELINED with a 1-stage
skew between groups, so every engine queue interleaves different stages of
different groups and nothing hard-barriers:
  S0: GT[d',q] += xn-chunk^T-stationary MMs;  GTs copy
  S1: F (512-col MM) + FT chunks;  ts4/s4/sq/ss (DVE);  tsTu = FT*maskT
  S2: newton-rsqrt scale [128,1];  flip to [1,q] (identity MM);  scB (K=1 MM)
  S3: tsTs = tsTu*scB;  HT += wtp^T MMs;  HTs copy
  S4: bt = xT-chunk MMs (i-part);  exp
  S5: z; rz; ct = e*rz (split DVE / GpSimd halves)
Final iter: S0, S1, then o = s*scale -> DMA out.
Sharding: data-parallel over batch, 16 samples/core x 8 cores.
"""

import os
import sys

sys.path.insert(0, "/opt/trn_rl_repo")

import numpy as np

import concourse.bass as bass
import concourse.bacc as bacc
import concourse.mybir as mybir
from concourse import tile
from concourse.bass_utils import run_bass_kernel_spmd

FP32 = mybir.dt.float32
BF16 = mybir.dt.bfloat16
I32 = mybir.dt.int32
AF = mybir.ActivationFunctionType
AX = mybir.AxisListType
AL = mybir.AluOpType

EPS = 1e-7
N_CORES = 8
B_TOTAL, IN, D = 128, 1024, 128
NCAP, DC = 32, 16
K = NCAP * DC
B_LOC = B_TOTAL // N_CORES
GSZ = 4
NG = B_LOC // GSZ
NCH = IN // 128


def build():
    nc = bacc.Bacc("TRN2", target_bir_lowering=False)
    xT = nc.declare_dram_parameter("xT", [B_LOC, D, IN], BF16, isOutput=False)
    xn = nc.declare_dram_parameter("xn", [B_LOC, 128, NCH, D], BF16, isOutput=False)
    wp = nc.declare_dram_parameter("wp", [D, K], BF16, isOutput=False)
    wpc = nc.declare_dram_parameter("wpc", [D, 4, 128], BF16, isOutput=False)
    wtp = nc.declare_dram_parameter("wtp", [K, D], BF16, isOutput=False)
    maskp = nc.declare_dram_parameter("maskp", [128, K], BF16, isOutput=False)
    maskt = nc.declare_dram_parameter("maskt", [128, 128], BF16, isOutput=False)
    ident = nc.declare_dram_parameter("ident", [128, 128], BF16, isOutput=False)
    out = nc.declare_dram_parameter("out", [B_LOC, NCAP, DC], FP32, isOutput=True)

    with tile.TileContext(nc) as tc:
        with (
            tc.tile_pool(name="const", bufs=1) as cpool,
            tc.tile_pool(name="xp", bufs=1) as xp,
            tc.tile_pool(name="sbp", bufs=4) as sbp,
            tc.tile_pool(name="tsp", bufs=4) as tsp,
            tc.tile_pool(name="ep", bufs=4) as ep,
            tc.tile_pool(name="ctp", bufs=8) as ctp,
            tc.tile_pool(name="small", bufs=16) as smallp,
            tc.tile_pool(name="gt", bufs=1, space="PSUM") as gtp,
            tc.tile_pool(name="fn", bufs=1, space="PSUM") as fnp,
            tc.tile_pool(name="ft", bufs=1, space="PSUM") as ftp,
            tc.tile_pool(name="sc", bufs=1, space="PSUM") as scp,
            tc.tile_pool(name="ht", bufs=1, space="PSUM") as htp,
            tc.tile_pool(name="bt", bufs=2, space="PSUM") as btp,
        ):
            wp_sb = cpool.tile([D, K], BF16, tag="wp")
            nc.sync.dma_start(wp_sb[:], wp[:])
            wpc_sb = cpool.tile([D, 4, 128], BF16, tag="wpc")
            nc.sync.dma_start(wpc_sb[:], wpc[:])
            wtp_sb = cpool.tile([128, 4, D], BF16, tag="wtp")
            nc.sync.dma_start(wtp_sb[:], wtp.rearrange("(j p) d -> p j d", p=128))
            mp_sb = cpool.tile([128, K], BF16, tag="maskp")
            nc.sync.dma_start(mp_sb[:], maskp[:])
            mt_sb = cpool.tile([128, 128], BF16, tag="maskt")
            nc.sync.dma_start(mt_sb[:], maskt[:])
            id_sb = cpool.tile([128, 128], BF16, tag="ident")
            nc.sync.dma_start(id_sb[:], ident[:])
            c0_sb = cpool.tile([128, NCAP], BF16, tag="c0")
            nc.vector.memset(c0_sb[:], 1.0 / NCAP)
            ones_row = cpool.tile([1, 128], BF16, tag="ones_row")
            nc.vector.memset(ones_row[:], 1.0)

            xT_g, xn_g = [], []
            for g in range(NG):
                t2 = xp.tile([128, GSZ, NCH, D], BF16, tag=f"xn{g}")
                t = xp.tile([128, GSZ, IN], BF16, tag=f"xT{g}")
                xn_g.append(t2)
                xT_g.append(t)
            # all xn first (feeds the G stages of every group early), then xT
            for g in range(NG):
                for b in range(GSZ):
                    nc.sync.dma_start(xn_g[g][:, b], xn[g * GSZ + b])
            for g in range(NG):
                for b in range(GSZ):
                    nc.sync.dma_start(xT_g[g][:, b, :], xT[g * GSZ + b])

            # persistent cross-stage state, per group
            st = [dict() for _ in range(NG)]
            ct = [None] * NG

            def s0(g, it):
                GT4 = gtp.tile([128, 128], FP32, tag="gt4")
                for b in range(GSZ):
                    for c in range(NCH):
                        mv = c0_sb[:] if it == 0 else ct[g][:, b, c, :]
                        nc.tensor.matmul(
                            GT4[:, 32 * b:32 * b + 32],
                            xn_g[g][:, b, c, :],
                            mv,
                            start=(c == 0),
                            stop=(c == NCH - 1),
                        )
                Gs = sbp.tile([128, 128], BF16, tag="gts")
                nc.scalar.copy(Gs[:], GT4[:])
                st[g]["GTs"] = Gs

            def s1(g, it):
                Gs = st[g].pop("GTs")
                F4t = fnp.tile([128, 4, 128], FP32, tag="f4n", name="F4n")
                F4 = F4t[:].rearrange("p j q -> p (j q)")
                nc.tensor.matmul(F4, Gs[:], wp_sb[:], start=True, stop=True)
                if it < 2:
                    FT4 = ftp.tile([128, 4, 128], FP32, tag="ft4t", name="FT4")
                    for j in range(4):
                        nc.tensor.matmul(
                            FT4[:, j, :], wpc_sb[:, j, :], Gs[:],
                            start=True, stop=True,
                        )
                    tsTu = tsp.tile([128, 4, 128], BF16, tag="tstu")
                    nc.vector.tensor_mul(
                        tsTu[:], FT4[:],
                        mt_sb[:].rearrange("p (a q) -> p a q", a=1)
                        .to_broadcast([128, 4, 128]),
                    )
                    st[g]["tsTu"] = tsTu
                ts4 = tsp.tile([128, K], BF16, tag="ts4")
                nc.vector.tensor_mul(ts4[:], F4, mp_sb[:])
                s4 = smallp.tile([128, DC], FP32, tag="s4")
                nc.vector.reduce_sum(
                    s4[:], ts4[:].rearrange("p (d n) -> p d n", d=DC), axis=AX.X
                )
                sq4 = smallp.tile([128, DC], FP32, tag="sq4")
                nc.vector.tensor_mul(sq4[:], s4[:], s4[:])
                ss4 = smallp.tile([128, 1], FP32, tag="ss4")
                nc.vector.reduce_sum(ss4[:], sq4[:], axis=AX.X)
                st[g]["s4"] = s4
                st[g]["ss4"] = ss4

            def s2(g, it):
                # newton-rsqrt scale, per-partition [128,1]
                ss4 = st[g].pop("ss4")
                p = smallp
                ve = p.tile([128, 1], FP32, tag="ve")
                nc.vector.tensor_scalar_add(ve[:], ss4[:], EPS)
                ib = p.tile([128, 1], I32, tag="ib")
                nc.vector.tensor_scalar(ib[:], ve[:].bitcast(I32), 1, None,
                                        op0=AL.arith_shift_right)
                nc.vector.tensor_scalar(ib[:], ib[:], -1, 0x5F3759DF,
                                        op0=AL.mult, op1=AL.add)
                y0 = ib[:].bitcast(FP32)
                aN = p.tile([128, 1], FP32, tag="aN")
                yN = p.tile([128, 1], FP32, tag="yN")
                nc.vector.tensor_mul(aN[:], y0, y0)
                nc.vector.tensor_mul(aN[:], aN[:], ve[:])
                nc.vector.tensor_scalar(aN[:], aN[:], -0.5, 1.5,
                                        op0=AL.mult, op1=AL.add)
                nc.vector.tensor_mul(yN[:], y0, aN[:])
                sv = p.tile([128, 1], FP32, tag="sv")
                nc.vector.tensor_mul(sv[:], yN[:], ve[:])
                den = p.tile([128, 1], FP32, tag="den")
                nc.vector.tensor_scalar_add(den[:], ve[:], 0.5)
                rden = p.tile([128, 1], FP32, tag="rden")
                nc.vector.reciprocal(rden[:], den[:])
                if it == 2:
                    sc4 = p.tile([128, 1], FP32, tag="sc4")
                    nc.vector.tensor_mul(sc4[:], sv[:], rden[:])
                    o4 = p.tile([128, DC], FP32, tag="o4")
                    nc.vector.tensor_scalar_mul(o4[:], st[g].pop("s4")[:],
                                                sc4[:])
                    nc.sync.dma_start(
                        out[g * GSZ:(g + 1) * GSZ].rearrange("b n d -> (b n) d"),
                        o4[:],
                    )
                    return
                st[g].pop("s4")
                sc4b = p.tile([128, 1], BF16, tag="sc4b")
                nc.vector.tensor_scalar_mul(sc4b[:], sv[:], rden[:])
                scT = scp.tile([1, 128], FP32, tag="sct", name="scTps")
                nc.tensor.matmul(scT[:], sc4b[:], id_sb[:], start=True,
                                 stop=True)
                scTs = p.tile([1, 128], BF16, tag="scTs")
                nc.scalar.copy(scTs[:], scT[:])
                scB = scp.tile([128, 128], FP32, tag="scb", name="scBps")
                nc.tensor.matmul(scB[:], ones_row[:], scTs[:], start=True,
                                 stop=True)
                scBs = sbp.tile([128, 128], BF16, tag="scbs")
                nc.scalar.copy(scBs[:], scB[:])
                st[g]["scBs"] = scBs

            def s3(g, it):
                tsTu = st[g].pop("tsTu")
                scBs = st[g].pop("scBs")
                tsTs = tsp.tile([128, 4, 128], BF16, tag="tsts")
                nc.vector.tensor_mul(
                    tsTs[:], tsTu[:],
                    scBs[:].rearrange("p (a q) -> p a q", a=1)
                    .to_broadcast([128, 4, 128]),
                )
                HTu = htp.tile([128, 128], FP32, tag="htu")
                for j in range(4):
                    nc.tensor.matmul(
                        HTu[:], wtp_sb[:, j, :], tsTs[:, j, :],
                        start=(j == 0), stop=(j == 3),
                    )
                HTs = sbp.tile([128, 128], BF16, tag="hts")
                nc.scalar.copy(HTs[:], HTu[:])
                st[g]["HTs"] = HTs

            def s4stage(g, it):
                HTs = st[g].pop("HTs")
                e4 = ep.tile([128, GSZ, NCH, NCAP], BF16, tag="e4")
                for h in range(2):
                    bt2 = btp.tile([128, 2, NCH, NCAP], FP32, tag="bt2")
                    for b2 in range(2):
                        b = 2 * h + b2
                        for c in range(NCH):
                            nc.tensor.matmul(
                                bt2[:, b2, c, :],
                                xT_g[g][:, b, 128 * c:128 * c + 128],
                                HTs[:, 32 * b:32 * b + 32],
                                start=True,
                                stop=True,
                            )
                    nc.scalar.activation(
                        e4[:, 2 * h:2 * h + 2].rearrange("p a c n -> p (a c n)"),
                        bt2[:].rearrange("p a c n -> p (a c n)"),
                        AF.Exp,
                    )
                st[g]["e4"] = e4

            def s5(g, it):
                e4 = st[g].pop("e4")
                z4 = smallp.tile([128, GSZ * NCH], FP32, tag="z4")
                nc.vector.reduce_sum(z4[:], e4[:], axis=AX.X)
                rz4 = smallp.tile([128, GSZ * NCH], BF16, tag="rz4")
                with nc.allow_low_precision("softmax denominators O(1-30)"):
                    nc.vector.reciprocal(rz4[:], z4[:])
                ctg = ctp.tile([128, GSZ, NCH, NCAP], BF16, tag="ct4")
                rzv = rz4[:].rearrange("p (b c) -> p b c", b=GSZ)
                nc.vector.tensor_mul(
                    ctg[:], e4[:],
                    rzv[:].to_broadcast([128, GSZ, NCH, NCAP]),
                )
                ct[g] = ctg

            # stage list per group: 3 iterations, last one truncated
            STAGES = []
            for it in range(2):
                STAGES += [(s0, it), (s1, it), (s2, it), (s3, it),
                           (s4stage, it), (s5, it)]
            STAGES += [(s0, 2), (s1, 2), (s2, 2)]

            NS = len(STAGES)
            for r in range(NS + NG - 1):
                for g in range(NG):
                    s = r - g
                    if 0 <= s < NS:
                        fn, it = STAGES[s]
                        fn(g, it)
    nc.compile()
    return nc


LAST_RESULT = None
_CONSTS = None


def _consts():
    global _CONSTS
    if _CONSTS is None:
        perm = np.empty(K, np.int64)
        for n in range(NCAP):
            for d in range(DC):
                perm[d * NCAP + n] = n * DC + d
        m32 = np.tile(np.eye(NCAP, dtype=np.float32), (1, DC)).reshape(NCAP, K)
        maskp = np.tile(m32, (GSZ, 1))
        pp, qq = np.meshgrid(np.arange(128), np.arange(128), indexing="ij")
        maskt = (pp % 32 == qq % 32).astype(np.float32)
        _CONSTS = (perm, maskp, maskt)
    return _CONSTS


def kernel(inputs, kernel):
    import ml_dtypes
    bf16 = ml_dtypes.bfloat16
    x = np.ascontiguousarray(np.asarray(inputs, dtype=np.float32))
    W = np.ascontiguousarray(np.asarray(kernel, dtype=np.float32)[0])
    xTh = np.ascontiguousarray(x.transpose(0, 2, 1).astype(bf16))
    xnL = np.ascontiguousarray(
        x.reshape(B_TOTAL, NCH, 128, D).transpose(0, 2, 1, 3).astype(bf16)
    )
    perm, maskp, maskt = _consts()
    WPf = W[:, perm]
    WP = np.ascontiguousarray(WPf.astype(bf16))
    WPC = np.ascontiguousarray(WPf.reshape(D, 4, 128).astype(bf16))
    WTP = np.ascontiguousarray(WPf.T.astype(bf16))

    nc = build()
    in_maps = [
        {
            "xT": xTh[i * B_LOC:(i + 1) * B_LOC],
            "xn": xnL[i * B_LOC:(i + 1) * B_LOC],
            "wp": WP,
            "wpc": WPC,
            "wtp": WTP,
            "maskp": maskp.astype(bf16),
            "maskt": maskt.astype(bf16),
            "ident": np.eye(128, dtype=np.float32).astype(bf16),
        }
        for i in range(N_CORES)
    ]
    res = run_bass_kernel_spmd(
        nc, in_maps, core_ids=list(range(N_CORES)),
        trace=bool(os.environ.get("KERNEL_TRACE")),
    )
    global LAST_RESULT
    LAST_RESULT = res
    return np.concatenate([res.results[i]["out"] for i in range(N_CORES)], axis=0)


if __name__ == "__main__":
    rng = np.random.default_rng(0)
    xi = rng.standard_normal((B_TOTAL, IN, D), dtype=np.float32)
    ki = (rng.standard_normal((1, D, K), dtype=np.float32) * 0.05).astype(np.float32)
    o = kernel(xi, ki)
    print(o.shape, o.dtype)


# revision 36
# speedup vs baseline: 1.2446x; 1.0162x over previous
"""Trainium2 Bass kernel for capsule dynamic routing (nn_Capsule).

Math (per sample):
  hat[i,(n,d)] = sum_d' x[i,d'] W[d',(n,d)]        (i=1024, d'=128, n=32, d=16)
  3 routing iters: c = softmax(b, axis=n); o = squash(sum_i c[n,i] hat[i,n,:])
                   b = sum_d o[n,d] hat[i,n,d]
Never materialize hat.  W columns permuted k' = d*32 + n so masked reduces are
contiguous and the mask is one [128,128] tile for every chunk.

Per group of 4 samples (stacked 4*32 = 128 partitions q=(b,n)) and iteration,
the work is split into 6 stages and EMITTED SOFTWARE-PIPELINED with a 1-stage
skew between groups, so every engine queue interleaves different stages of
different groups and nothing hard-barriers:
  S0: GT[d',q] += xn-chunk^T-stationary MMs;  GTs copy
  S1: F (512-col MM) + FT chunks;  ts4/s4/sq/ss (DVE);  tsTu = FT*maskT
  S2: newton-rsqrt scale [128,1];  flip to [1,q] (identity MM);  scB (K=1 MM)
  S3: tsTs = tsTu*scB;  HT += wtp^T MMs;  HTs copy
  S4: bt = xT-chunk MMs (i-part);  exp
  S5: z; rz; ct = e*rz (split DVE / GpSimd halves)
Final iter: S0, S1, then o = s*scale -> DMA out.
Sharding: data-parallel over batch, 16 samples/core x 8 cores.
"""

import os
import sys

sys.path.insert(0, "/opt/trn_rl_repo")

import numpy as np

import concourse.bass as bass
import concourse.bacc as bacc
import concourse.mybir as mybir
from concourse import tile
from concourse.bass_utils import run_bass_kernel_spmd

FP32 = mybir.dt.float32
BF16 = mybir.dt.bfloat16
I32 = mybir.dt.int32
AF = mybir.ActivationFunctionType
AX = mybir.AxisListType
AL = mybir.AluOpType

EPS = 1e-7
N_CORES = 8
B_TOTAL, IN, D = 128, 1024, 128
NCAP, DC = 32, 16
K = NCAP * DC
B_LOC = B_TOTAL // N_CORES
GSZ = 4
NG = B_LOC // GSZ
NCH = IN // 128


def build():
    nc = bacc.Bacc("TRN2", target_bir_lowering=False)
    xT = nc.declare_dram_parameter("xT", [B_LOC, D, IN], BF16, isOutput=False)
    xn = nc.declare_dram_parameter("xn", [B_LOC, 128, NCH, D], BF16, isOutput=False)
    wp = nc.declare_dram_parameter("wp", [D, K], BF16, isOutput=False)
    wpc = nc.declare_dram_parameter("wpc", [D, 4, 128], BF16, isOutput=False)
    wtp = nc.declare_dram_parameter("wtp", [K, D], BF16, isOutput=False)
    maskp = nc.declare_dram_parameter("maskp", [128, K], BF16, isOutput=False)
    maskt = nc.declare_dram_parameter("maskt", [128, 128], BF16, isOutput=False)
    ident = nc.declare_dram_parameter("ident", [128, 128], BF16, isOutput=False)
    out = nc.declare_dram_parameter("out", [B_LOC, NCAP, DC], FP32, isOutput=True)

    with tile.TileContext(nc) as tc:
        with (
            tc.tile_pool(name="const", bufs=1) as cpool,
            tc.tile_pool(name="xp", bufs=1) as xp,
            tc.tile_pool(name="sbp", bufs=4) as sbp,
            tc.tile_pool(name="tsp", bufs=4) as tsp,
            tc.tile_pool(name="ep", bufs=4) as ep,
            tc.tile_pool(name="ctp", bufs=8) as ctp,
            tc.tile_pool(name="small", bufs=16) as smallp,
            tc.tile_pool(name="gt", bufs=1, space="PSUM") as gtp,
            tc.tile_pool(name="fn", bufs=1, space="PSUM") as fnp,
            tc.tile_pool(name="ft", bufs=1, space="PSUM") as ftp,
            tc.tile_pool(name="sc", bufs=1, space="PSUM") as scp,
            tc.tile_pool(name="ht", bufs=1, space="PSUM") as htp,
            tc.tile_pool(name="bt", bufs=2, space="PSUM") as btp,
        ):
            # xn group 0 first so the pipeline ramps immediately
            xn_t = []
            for g in range(NG):
                t2 = xp.tile([128, GSZ, NCH, D], BF16, tag=f"xn{g}",
                             name=f"xng{g}")
                xn_t.append(t2)
            for b in range(GSZ):
                nc.sync.dma_start(xn_t[0][:, b], xn[b])
            wp_sb = cpool.tile([D, K], BF16, tag="wp")
            nc.sync.dma_start(wp_sb[:], wp[:])
            wpc_sb = cpool.tile([D, 4, 128], BF16, tag="wpc")
            nc.sync.dma_start(wpc_sb[:], wpc[:])
            wtp_sb = cpool.tile([128, 4, D], BF16, tag="wtp")
            nc.sync.dma_start(wtp_sb[:], wtp.rearrange("(j p) d -> p j d", p=128))
            mp_sb = cpool.tile([128, K], BF16, tag="maskp")
            nc.sync.dma_start(mp_sb[:], maskp[:])
            mt_sb = cpool.tile([128, 128], BF16, tag="maskt")
            nc.sync.dma_start(mt_sb[:], maskt[:])
            id_sb = cpool.tile([128, 128], BF16, tag="ident")
            nc.sync.dma_start(id_sb[:], ident[:])
            c0_sb = cpool.tile([128, NCAP], BF16, tag="c0")
            nc.vector.memset(c0_sb[:], 1.0 / NCAP)
            ones_row = cpool.tile([1, 128], BF16, tag="ones_row")
            nc.vector.memset(ones_row[:], 1.0)

            xn_g = xn_t
            xT_g = []
            for g in range(NG):
                t = xp.tile([128, GSZ, IN], BF16, tag=f"xT{g}", name=f"xTg{g}")
                xT_g.append(t)
            # remaining xn (group 0 already queued above), then all xT
            for g in range(1, NG):
                for b in range(GSZ):
                    nc.sync.dma_start(xn_g[g][:, b], xn[g * GSZ + b])
            for g in range(NG):
                for b in range(GSZ):
                    nc.sync.dma_start(xT_g[g][:, b, :], xT[g * GSZ + b])

            # persistent cross-stage state, per group
            st = [dict() for _ in range(NG)]
            ct = [None] * NG

            def s0(g, it):
                GT4 = gtp.tile([128, 128], FP32, tag="gt4")
                for b in range(GSZ):
                    for c in range(NCH):
                        mv = c0_sb[:] if it == 0 else ct[g][:, b, c, :]
                        nc.tensor.matmul(
                            GT4[:, 32 * b:32 * b + 32],
                            xn_g[g][:, b, c, :],
                            mv,
                            start=(c == 0),
                            stop=(c == NCH - 1),
                        )
                Gs = sbp.tile([128, 128], BF16, tag="gts")
                nc.scalar.copy(Gs[:], GT4[:])
                st[g]["GTs"] = Gs

            def s1(g, it):
                Gs = st[g].pop("GTs")
                F4t = fnp.tile([128, 4, 128], FP32, tag="f4n", name="F4n")
                F4 = F4t[:].rearrange("p j q -> p (j q)")
                nc.tensor.matmul(F4, Gs[:], wp_sb[:], start=True, stop=True)
                if it < 2:
                    FT4 = ftp.tile([128, 4, 128], FP32, tag="ft4t", name="FT4")
                    for j in range(4):
                        nc.tensor.matmul(
                            FT4[:, j, :], wpc_sb[:, j, :], Gs[:],
                            start=True, stop=True,
                        )
                    tsTu = tsp.tile([128, 4, 128], BF16, tag="tstu")
                    nc.vector.tensor_mul(
                        tsTu[:], FT4[:],
                        mt_sb[:].rearrange("p (a q) -> p a q", a=1)
                        .to_broadcast([128, 4, 128]),
                    )
                    st[g]["tsTu"] = tsTu
                ts4 = tsp.tile([128, K], BF16, tag="ts4")
                nc.vector.tensor_mul(ts4[:], F4, mp_sb[:])
                ss4 = smallp.tile([128, 1], FP32, tag="ss4")
                if it == 2:
                    s4 = smallp.tile([128, DC], FP32, tag="s4")
                    nc.vector.reduce_sum(
                        s4[:], ts4[:].rearrange("p (d n) -> p d n", d=DC),
                        axis=AX.X,
                    )
                    sq4 = smallp.tile([128, DC], FP32, tag="sq4")
                    nc.vector.tensor_mul(sq4[:], s4[:], s4[:])
                    nc.vector.reduce_sum(ss4[:], sq4[:], axis=AX.X)
                    st[g]["s4"] = s4
                else:
                    # ss = sum((F*mask)^2): masked rows hold s_d exactly once
                    dead = tsp.tile([128, K], BF16, tag="dead")
                    nc.scalar.activation(dead[:], ts4[:], AF.Square,
                                         accum_out=ss4[:])
                st[g]["ss4"] = ss4

            def s2(g, it):
                # newton-rsqrt scale, per-partition [128,1]
                ss4 = st[g].pop("ss4")
                p = smallp
                ve = p.tile([128, 1], FP32, tag="ve")
                nc.vector.tensor_scalar_add(ve[:], ss4[:], EPS)
                ib = p.tile([128, 1], I32, tag="ib")
                nc.vector.tensor_scalar(ib[:], ve[:].bitcast(I32), 1, None,
                                        op0=AL.arith_shift_right)
                nc.vector.tensor_scalar(ib[:], ib[:], -1, 0x5F3759DF,
                                        op0=AL.mult, op1=AL.add)
                y0 = ib[:].bitcast(FP32)
                aN = p.tile([128, 1], FP32, tag="aN")
                yN = p.tile([128, 1], FP32, tag="yN")
                nc.vector.tensor_mul(aN[:], y0, y0)
                nc.vector.tensor_mul(aN[:], aN[:], ve[:])
                nc.vector.tensor_scalar(aN[:], aN[:], -0.5, 1.5,
                                        op0=AL.mult, op1=AL.add)
                nc.vector.tensor_mul(yN[:], y0, aN[:])
                sv = p.tile([128, 1], FP32, tag="sv")
                nc.vector.tensor_mul(sv[:], yN[:], ve[:])
                den = p.tile([128, 1], FP32, tag="den")
                nc.vector.tensor_scalar_add(den[:], ve[:], 0.5)
                rden = p.tile([128, 1], FP32, tag="rden")
                nc.vector.reciprocal(rden[:], den[:])
                if it == 2:
                    sc4 = p.tile([128, 1], FP32, tag="sc4")
                    nc.vector.tensor_mul(sc4[:], sv[:], rden[:])
                    o4 = p.tile([128, DC], FP32, tag="o4")
                    nc.vector.tensor_scalar_mul(o4[:], st[g].pop("s4")[:],
                                                sc4[:])
                    nc.sync.dma_start(
                        out[g * GSZ:(g + 1) * GSZ].rearrange("b n d -> (b n) d"),
                        o4[:],
                    )
                    return
                sc4b = p.tile([128, 1], BF16, tag="sc4b")
                nc.vector.tensor_scalar_mul(sc4b[:], sv[:], rden[:])
                scT = scp.tile([1, 128], FP32, tag="sct", name="scTps")
                nc.tensor.matmul(scT[:], sc4b[:], id_sb[:], start=True,
                                 stop=True)
                scTs = p.tile([1, 128], BF16, tag="scTs")
                nc.scalar.copy(scTs[:], scT[:])
                scB = scp.tile([128, 128], FP32, tag="scb", name="scBps")
                nc.tensor.matmul(scB[:], ones_row[:], scTs[:], start=True,
                                 stop=True)
                scBs = sbp.tile([128, 128], BF16, tag="scbs")
                nc.scalar.copy(scBs[:], scB[:])
                st[g]["scBs"] = scBs

            def s3(g, it):
                tsTu = st[g].pop("tsTu")
                scBs = st[g].pop("scBs")
                tsTs = tsp.tile([128, 4, 128], BF16, tag="tsts")
                nc.vector.tensor_mul(
                    tsTs[:], tsTu[:],
                    scBs[:].rearrange("p (a q) -> p a q", a=1)
                    .to_broadcast([128, 4, 128]),
                )
                HTu = htp.tile([128, 128], FP32, tag="htu")
                for j in range(4):
                    nc.tensor.matmul(
                        HTu[:], wtp_sb[:, j, :], tsTs[:, j, :],
                        start=(j == 0), stop=(j == 3),
                    )
                HTs = sbp.tile([128, 128], BF16, tag="hts")
                nc.scalar.copy(HTs[:], HTu[:])
                st[g]["HTs"] = HTs

            def s4stage(g, it):
                HTs = st[g].pop("HTs")
                e4 = ep.tile([128, GSZ, NCH, NCAP], BF16, tag="e4")
                for h in range(2):
                    bt2 = btp.tile([128, 2, NCH, NCAP], FP32, tag="bt2")
                    for b2 in range(2):
                        b = 2 * h + b2
                        for c in range(NCH):
                            nc.tensor.matmul(
                                bt2[:, b2, c, :],
                                xT_g[g][:, b, 128 * c:128 * c + 128],
                                HTs[:, 32 * b:32 * b + 32],
                                start=True,
                                stop=True,
                            )
                    nc.scalar.activation(
                        e4[:, 2 * h:2 * h + 2].rearrange("p a c n -> p (a c n)"),
                        bt2[:].rearrange("p a c n -> p (a c n)"),
                        AF.Exp,
                    )
                st[g]["e4"] = e4

            def s5(g, it):
                e4 = st[g].pop("e4")
                z4 = smallp.tile([128, GSZ * NCH], FP32, tag="z4")
                nc.vector.reduce_sum(z4[:], e4[:], axis=AX.X)
                rz4 = smallp.tile([128, GSZ * NCH], BF16, tag="rz4")
                with nc.allow_low_precision("softmax denominators O(1-30)"):
                    nc.vector.reciprocal(rz4[:], z4[:])
                ctg = ctp.tile([128, GSZ, NCH, NCAP], BF16, tag="ct4")
                rzv = rz4[:].rearrange("p (b c) -> p b c", b=GSZ)
                nc.vector.tensor_mul(
                    ctg[:, 0:2], e4[:, 0:2],
                    rzv[:, 0:2].to_broadcast([128, 2, NCH, NCAP]),
                )
                nc.gpsimd.tensor_mul(
                    ctg[:, 2:4], e4[:, 2:4],
                    rzv[:, 2:4].to_broadcast([128, 2, NCH, NCAP]),
                )
                ct[g] = ctg

            # stage list per group: 3 iterations, last one truncated
            STAGES = []
            for it in range(2):
                STAGES += [(s0, it), (s1, it), (s2, it), (s3, it),
                           (s4stage, it), (s5, it)]
            STAGES += [(s0, 2), (s1, 2), (s2, 2)]

            NS = len(STAGES)
            for r in range(NS + NG - 1):
                for g in range(NG):
                    s = r - g
                    if 0 <= s < NS:
                        fn, it = STAGES[s]
                        fn(g, it)
    nc.compile()
    return nc


LAST_RESULT = None
_CONSTS = None


def _consts():
    global _CONSTS
    if _CONSTS is None:
        perm = np.empty(K, np.int64)
        for n in range(NCAP):
            for d in range(DC):
                perm[d * NCAP + n] = n * DC + d
        m32 = np.tile(np.eye(NCAP, dtype=np.float32), (1, DC)).reshape(NCAP, K)
        maskp = np.tile(m32, (GSZ, 1))
        pp, qq = np.meshgrid(np.arange(128), np.arange(128), indexing="ij")
        maskt = (pp % 32 == qq % 32).astype(np.float32)
        _CONSTS = (perm, maskp, maskt)
    return _CONSTS


def kernel(inputs, kernel):
    import ml_dtypes
    bf16 = ml_dtypes.bfloat16
    x = np.ascontiguousarray(np.asarray(inputs, dtype=np.float32))
    W = np.ascontiguousarray(np.asarray(kernel, dtype=np.float32)[0])
    xTh = np.ascontiguousarray(x.transpose(0, 2, 1).astype(bf16))
    xnL = np.ascontiguousarray(
        x.reshape(B_TOTAL, NCH, 128, D).transpose(0, 2, 1, 3).astype(bf16)
    )
    perm, maskp, maskt = _consts()
    WPf = W[:, perm]
    WP = np.ascontiguousarray(WPf.astype(bf16))
    WPC = np.ascontiguousarray(WPf.reshape(D, 4, 128).astype(bf16))
    WTP = np.ascontiguousarray(WPf.T.astype(bf16))

    nc = build()
    in_maps = [
        {
            "xT": xTh[i * B_LOC:(i + 1) * B_LOC],
            "xn": xnL[i * B_LOC:(i + 1) * B_LOC],
            "wp": WP,
            "wpc": WPC,
            "wtp": WTP,
            "maskp": maskp.astype(bf16),
            "maskt": maskt.astype(bf16),
            "ident": np.eye(128, dtype=np.float32).astype(bf16),
        }
        for i in range(N_CORES)
    ]
    res = run_bass_kernel_spmd(
        nc, in_maps, core_ids=list(range(N_CORES)),
        trace=bool(os.environ.get("KERNEL_TRACE")),
    )
    global LAST_RESULT
    LAST_RESULT = res
    return np.concatenate([res.results[i]["out"] for i in range(N_CORES)], axis=0)


if __name__ == "__main__":
    rng = np.random.default_rng(0)
    xi = rng.standard_normal((B_TOTAL, IN, D), dtype=np.float32)
    ki = (rng.standard_normal((1, D, K), dtype=np.float32) * 0.05).astype(np.float32)
    o = kernel(xi, ki)
    print(o.shape, o.dtype)
